# revision 12
# baseline (speedup 1.0000x reference)
"""EnhancedGovernanceAttention Trainium2 kernel (8 NeuronCores, SPMD).

Sharding: core c owns heads {2c, 2c+1} for BOTH batches (policy_mask is
per-head and batch-shared, so each policy slice is loaded once per core
and reused for both batches).  Each core computes its heads' attention
and a row-parallel partial of the Wo projection; the host sums the 8
bf16 partials (fp32 accumulate) and adds bo.

Math notes (vs the jax reference):
 - softmax max-subtraction is dropped: scores ~ N(0,1) + small bias, so
   exp() cannot overflow in fp32; softmax is shift-invariant.
 - log1p memory bias: log(w) with w = 1 + GS*mw + 1e-8 is applied as the
   per-partition (k) bias operand of the exp activation, so
   ex = w * exp(s); the denominator is then a plain ones-row matmul.
 - the policy bias (with the causal -40 mask baked into the diagonal
   tiles) is added to the scores on the PE via a bf16 identity matmul
   accumulating into the same psum group as the k.q matmul.
 - scores are computed TRANSPOSED ([k, q]) so the PV matmul directly
   yields attn^T, which is the lhsT the output projection needs.
 - attn^T is normalized out of psum by a DVE multiply with the
   partition-broadcast reciprocal of the denominator.
 - the whole PE pipeline runs in bf16 (x, W, q/k/v, exp, attn, Wo);
   psum accumulation stays fp32.
"""

import numpy as np
import ml_dtypes
from contextlib import ExitStack

import concourse.bass as bass
import concourse.tile as tile
from concourse import bacc, mybir
from concourse.bass_utils import run_bass_kernel_spmd
from concourse.masks import make_identity

B, S, D, H, HD = 2, 2048, 2048, 16, 128
GS = 0.1
ROPE_BASE = 10000.0
NCORES = 8
HPC = H // NCORES          # heads per core = 2
SCALE = float(HD) ** -0.5
DT = D // 128              # 16 d-tiles
ST = S // 128              # 16 s-tiles (also k-tiles)
QB = 512                   # q-block width (phase B)
NQB = S // QB              # 4 q-blocks
SB = 256                   # s-block width (phase A panels)
NSB = S // SB              # 8 s-blocks
MASK_NEG = -40.0
SLAB_K = 2                 # k-tiles per bias slab load

F32 = mybir.dt.float32
BF16 = mybir.dt.bfloat16

_CACHE = {}


def build_nc():
    nc = bacc.Bacc("TRN2", target_bir_lowering=False, debug=False,
                   num_devices=NCORES)

    d_x = nc.dram_tensor("xbf", [B, S, D], BF16, kind="ExternalInput").ap()
    d_wq = nc.dram_tensor("wq", [D, HPC * HD], BF16, kind="ExternalInput").ap()
    d_wk = nc.dram_tensor("wk", [D, HPC * HD], BF16, kind="ExternalInput").ap()
    d_wv = nc.dram_tensor("wv", [D, HPC * HD], BF16, kind="ExternalInput").ap()
    d_wo = nc.dram_tensor("wo", [HPC * HD, D], BF16, kind="ExternalInput").ap()
    d_bias = nc.dram_tensor("biasT", [HPC, S, S], BF16, kind="ExternalInput").ap()
    d_logw = nc.dram_tensor("logw", [B, S], F32, kind="ExternalInput").ap()
    d_csc = nc.dram_tensor("csc", [128, S], BF16, kind="ExternalInput").ap()
    d_csn = nc.dram_tensor("csn", [128, S], BF16, kind="ExternalInput").ap()
    d_y = nc.dram_tensor("y", [B, S, D], BF16, kind="ExternalOutput").ap()

    with tile.TileContext(nc) as tc, ExitStack() as ctx:
        consts = ctx.enter_context(tc.tile_pool(name="consts", bufs=1))
        wpool = ctx.enter_context(tc.tile_pool(name="wpool", bufs=1))
        qkv = ctx.enter_context(tc.tile_pool(name="qkv", bufs=1))
        panels = ctx.enter_context(tc.tile_pool(name="panels", bufs=3))
        rope = ctx.enter_context(tc.tile_pool(name="rope", bufs=2))
        slabs = ctx.enter_context(tc.tile_pool(name="slabs", bufs=4))
        expp = ctx.enter_context(tc.tile_pool(name="expp", bufs=6))
        normp = ctx.enter_context(tc.tile_pool(name="normp", bufs=2))
        outp = ctx.enter_context(tc.tile_pool(name="outp", bufs=4))
        # PSUM budget is 8 banks of [128 x 2KB]:
        #   psA (4): bias+score tiles, phase-C matmul tiles, phase-A qkv tiles
        #   psB (2): pv accumulators (b0, b1)
        #   psL (2): denominator accumulators + transposed-reciprocal tiles
        psA = ctx.enter_context(tc.tile_pool(name="psA", bufs=4, space="PSUM"))
        psB = ctx.enter_context(tc.tile_pool(name="psB", bufs=2, space="PSUM"))
        psL = ctx.enter_context(tc.tile_pool(name="psL", bufs=2, space="PSUM"))

        # ------------- constants (emission order = DMA priority) -------------
        # Everything startup-critical goes on the SP HWDGE queue in the exact
        # order the DMA device should serve it; wv/logw/wo are emitted inside
        # the phase-A loop so panel transposes interleave ahead of them.
        t_w = {}
        for name, dram in (("wq", d_wq), ("wk", d_wk)):
            t = wpool.tile([128, DT, HPC * HD], BF16, tag=name, name=name)
            nc.sync.dma_start(t, dram.rearrange("(t p) c -> p t c", p=128))
            t_w[name] = t
        t_w["wv"] = wpool.tile([128, DT, HPC * HD], BF16, tag="wv", name="wv")
        t_csc = consts.tile([128, S], BF16, tag="csc")
        t_csn = consts.tile([128, S], BF16, tag="csn")

        ones_bf = consts.tile([128, 1], BF16, tag="ones_bf")
        nc.gpsimd.memset(ones_bf, 1.0)
        ident_bf = consts.tile([128, 128], BF16, tag="ident_bf")
        make_identity(nc, ident_bf)
        t_logw = consts.tile([128, B, ST], F32, tag="logw")
        t_wo = consts.tile([128, HPC, D], BF16, tag="wo")

        # y writes are issued one C-unit late so the DMA's wait on the ob
        # copy is already satisfied when it reaches the issuing sequencer
        # (an unsatisfied wait would hold the SEQ and block later issues).
        y_lag = []

        def flush_y(nmax=1):
            for _ in range(min(nmax, len(y_lag))):
                eng, dst, ob = y_lag.pop(0)
                eng.dma_start(dst, ob)

        def emit_c_unit(b_ref, st, nb):
            ss = slice(st * 128, (st + 1) * 128)
            ns = slice(nb * 512, (nb + 1) * 512)
            ops = psA.tile([128, 512], F32, tag="mm", name="ops")
            for h in range(HPC):
                nc.tensor.matmul(
                    ops, attnT[b_ref, h][:, ss], t_wo[:, h, ns],
                    start=(h == 0), stop=(h == HPC - 1))
            ob = outp.tile([128, 512], BF16, tag="ob")
            ceng = nc.vector if (st + nb) % 2 == 0 else nc.scalar
            ceng.tensor_copy(ob, ops) if ceng is nc.vector else ceng.copy(ob, ops)
            eng = nc.scalar if (st + nb) % 2 == 0 else nc.sync
            y_lag.append((eng, d_y[b_ref, ss, ns], ob))
            flush_y(1) if len(y_lag) > 2 else None

        # ================= phase A: QKV + RoPE for both batches ==============
        qT = {}
        kT = {}
        vv = {}
        for b in range(B):
            for h in range(HPC):
                qT[b, h] = qkv.tile([128, S], BF16, tag=f"qT{b}{h}", name=f"qT{b}{h}")
                kT[b, h] = qkv.tile([128, S], BF16, tag=f"kT{b}{h}", name=f"kT{b}{h}")
            vv[b] = qkv.tile([128, ST, HPC * HD], BF16, tag=f"v{b}", name=f"v{b}")

        for b in range(B):
            for sb_i in range(NSB):
                s0 = sb_i * SB
                blk = slice(s0, s0 + SB)
                panel = panels.tile([128, DT, SB], BF16, tag="panel", name="panel")
                half = DT // 2
                nc.sync.dma_start_transpose(
                    panel[:, :half, :], d_x[b, blk, :half * 128])
                nc.sync.dma_start_transpose(
                    panel[:, half:, :], d_x[b, blk, half * 128:])
                if b == 0 and sb_i == 0:
                    nc.sync.dma_start(t_csc[:, :S // 2], d_csc[:, :S // 2])
                    nc.sync.dma_start(t_csn[:, :S // 2], d_csn[:, :S // 2])
                if b == 0 and sb_i == 1:
                    nc.sync.dma_start(
                        t_w["wv"], d_wv.rearrange("(t p) c -> p t c", p=128))
                if b == 0 and sb_i == 2:
                    nc.sync.dma_start(
                        t_logw, d_logw.rearrange("b (t p) -> p b t", p=128))
                    nc.sync.dma_start(
                        t_wo, d_wo.rearrange("(h p) c -> p h c", p=128))
                if b == 0 and sb_i == 3:
                    nc.sync.dma_start(t_csc[:, S // 2:], d_csc[:, S // 2:])
                    nc.sync.dma_start(t_csn[:, S // 2:], d_csn[:, S // 2:])

                for h in range(HPC):
                    hc = slice(h * HD, (h + 1) * HD)
                    # --- q^T and k^T with fused RoPE ---
                    for name, dest in (("wq", qT[b, h]), ("wk", kT[b, h])):
                        ps = psA.tile([128, SB], F32, tag="mm")
                        for dt in range(DT):
                            nc.tensor.matmul(
                                ps, t_w[name][:, dt, hc], panel[:, dt, :],
                                start=(dt == 0), stop=(dt == DT - 1))
                        # RoPE: dest = ps * [cos;cos] + swap(ps) * [-sin;sin]
                        t1 = rope.tile([128, SB], F32, tag="t1")
                        t2 = rope.tile([128, SB], F32, tag="t2")
                        nc.vector.tensor_mul(t1, ps, t_csc[:, blk])
                        nc.vector.tensor_mul(
                            t2[0:64, :], ps[64:128, :], t_csn[0:64, blk])
                        nc.vector.tensor_mul(
                            t2[64:128, :], ps[0:64, :], t_csn[64:128, blk])
                        nc.gpsimd.tensor_add(dest[:, blk], t1, t2)
                # --- v in natural [s, hd] layout (both heads at once) ---
                for c4 in range(SB // 128):
                    stile = sb_i * (SB // 128) + c4
                    ch = slice(c4 * 128, (c4 + 1) * 128)
                    psv = psA.tile([128, HPC * HD], F32, tag="mm")
                    for dt in range(DT):
                        nc.tensor.matmul(
                            psv, panel[:, dt, ch], t_w["wv"][:, dt, :],
                            start=(dt == 0), stop=(dt == DT - 1))
                    nc.scalar.copy(vv[b][:, stile, :], psv)

        # ========== phases B+C software-pipelined over q-blocks ==========
        # attnT (normalized) overwrites qT[b,h][:, qs] after its last read
        attnT = {(b, h): qT[b, h] for b in range(B) for h in range(HPC)}
        pending_c = []
        for j in range(NQB):
            qs = slice(j * QB, (j + 1) * QB)
            nk = 4 * (j + 1)          # causal: k-tiles 0..nk-1
            for h in range(HPC):
                steps_left = nk * B
                pv = {b: psB.tile([128, QB], F32, tag="pv", name=f"pv{b}")
                      for b in range(B)}
                lps = {b: psL.tile([1, QB], F32, tag="l", name=f"l{b}")
                       for b in range(B)}
                for g in range((nk + SLAB_K - 1) // SLAB_K):
                  n = min(SLAB_K, nk - g * SLAB_K)
                  slab = slabs.tile([128, SLAB_K, QB], BF16, tag="slab")
                  k0 = g * SLAB_K * 128
                  nc.sync.dma_start(
                      slab[:, :n, :],
                      d_bias[h, k0:k0 + n * 128, qs].rearrange(
                          "(m p) q -> p m q", p=128))
                  for ml in range(n):
                    m = g * SLAB_K + ml
                    # columns q < 128*m are fully causal-masked; skip them
                    off = max(0, (m - 4 * j) * 128)
                    qso = slice(j * QB + off, (j + 1) * QB)
                    exs = {}
                    for b in range(B):
                        sc = psA.tile([128, QB], F32, tag="mm")
                        nc.tensor.matmul(
                            sc[:, off:], kT[b, h][:, m * 128:(m + 1) * 128],
                            qT[b, h][:, qso],
                            start=True, stop=False)
                        nc.tensor.matmul(
                            sc[:, off:], ident_bf, slab[:, ml, off:],
                            start=False, stop=True, skip_group_check=True)
                        ex = expp.tile([128, QB], BF16, tag="ex")
                        nc.scalar.activation(
                            ex[:, off:], sc[:, off:],
                            mybir.ActivationFunctionType.Exp,
                            bias=t_logw[:, b, m:m + 1])
                        exs[b] = ex
                    for b in range(B):
                        nc.tensor.matmul(
                            pv[b][:, off:], vv[b][:, m, h * HD:(h + 1) * HD],
                            exs[b][:, off:],
                            start=(m == 0), stop=(m == nk - 1),
                            skip_group_check=True)
                        nc.tensor.matmul(
                            lps[b][:, off:], ones_bf, exs[b][:, off:],
                            start=(m == 0), stop=(m == nk - 1),
                            skip_group_check=True)
                    # interleave pending output-projection units
                    npop = 2 if len(pending_c) > 16 else 1
                    for _ in range(npop):
                        if pending_c and (steps_left <= len(pending_c)
                                          or (m + h) % 2 == 0 or npop > 1):
                            emit_c_unit(*pending_c.pop(0))
                    steps_left -= 1
                for b in range(B):
                    rl = normp.tile([1, QB], F32, tag="rl")
                    nc.vector.reciprocal(rl, lps[b])
                    rb = normp.tile([128, QB], F32, tag="rb")
                    nc.gpsimd.partition_broadcast(rb, rl)
                    nc.vector.tensor_mul(attnT[b, h][:, qs], pv[b], rb)
            pending_c = pending_c + [
                (b, st, nb) for st in range(4 * j, 4 * j + 4)
                for b in range(B) for nb in range(D // 512)]
        for c in pending_c:
            emit_c_unit(*c)
        flush_y(len(y_lag))

    nc.compile()
    return nc


def _host_prep(x, Wq, Wk, Wv, Wo, policy_mask, memory_weights):
    """Build the per-core input maps."""
    bf = ml_dtypes.bfloat16
    xbf = np.asarray(x, dtype=bf)

    # RoPE tables, transposed: csc = [cosT; cosT], csn = [-sinT; sinT]
    inv_freq = (1.0 / (ROPE_BASE ** (np.arange(0, HD, 2, dtype=np.float32) / HD)))
    t = np.arange(S, dtype=np.float32)
    freqs = np.outer(t, inv_freq).astype(np.float32)      # [S, 64]
    cosT = np.cos(freqs).T.astype(np.float32)             # [64, S]
    sinT = np.sin(freqs).T.astype(np.float32)
    csc = np.ascontiguousarray(np.concatenate([cosT, cosT], axis=0)).astype(bf)
    csn = np.ascontiguousarray(np.concatenate([-sinT, sinT], axis=0)).astype(bf)

    # memory multiplier w = 1 + GS*mw + 1e-8  (exp(log1p(z)) = 1+z)
    mw = memory_weights.reshape(B, S).astype(np.float64)
    logw = np.log(1.0 + GS * mw + 1e-8).astype(np.float32)

    # transposed, causal-masked, pre-scaled policy bias per head (bf16)
    maskT = np.tril(np.full((S, S), MASK_NEG, dtype=np.float32), -1)
    pol = np.asarray(policy_mask, dtype=np.float32)[0]    # [H, S, S]

    in_maps = []
    for c in range(NCORES):
        cols = slice(c * HPC * HD, (c + 1) * HPC * HD)
        bias_c = np.empty((HPC, S, S), dtype=bf)
        for hl in range(HPC):
            hg = c * HPC + hl
            bias_c[hl] = (GS * pol[hg].T + maskT).astype(bf)
        in_maps.append({
            "xbf": xbf,
            "wq": np.ascontiguousarray(Wq[:, cols]).astype(bf),
            "wk": np.ascontiguousarray(Wk[:, cols] * np.float32(SCALE)).astype(bf),
            "wv": np.ascontiguousarray(Wv[:, cols]).astype(bf),
            "wo": np.ascontiguousarray(Wo[cols, :]).astype(bf),
            "biasT": bias_c,
            "logw": logw,
            "csc": csc, "csn": csn,
        })
    return in_maps


def kernel(x, Wq, Wk, Wv, Wo, bo, policy_mask, memory_weights):
    x = np.asarray(x, dtype=np.float32)
    Wq = np.asarray(Wq, dtype=np.float32)
    Wk = np.asarray(Wk, dtype=np.float32)
    Wv = np.asarray(Wv, dtype=np.float32)
    Wo = np.asarray(Wo, dtype=np.float32)
    bo = np.asarray(bo, dtype=np.float32)

    if "nc" not in _CACHE:
        _CACHE["nc"] = build_nc()
    nc = _CACHE["nc"]

    in_maps = _host_prep(x, Wq, Wk, Wv, Wo, policy_mask, memory_weights)
    res = run_bass_kernel_spmd(nc, in_maps, core_ids=list(range(NCORES)))

    acc = np.zeros((B, S, D), dtype=np.float64)
    for c in range(NCORES):
        acc += res.results[c]["y"].astype(np.float64)
    return (acc + bo.astype(np.float64)).astype(np.float32)
